# revision 19
# baseline (speedup 1.0000x reference)
"""Trainium2 Bass kernel for nn_DampedIMEX1Layer (v4).

Math: the per-step 2x2 transition M (per diagonal state p) is constant over
time, so the associative scan is a constant-coefficient linear recurrence.
Per core (= one batch element, data-parallel over 8 cores), chunk T=8 over
L=8192 (C=1024 chunks):

  extract  hhat streams (comp k, re/im) with V^{-1}-folded weights, fp8
           DoubleRow j-pairs (PE); q-streams {0,2} first so the ri=0 chain
           starts early; a short junk warmup spins the PE HAM clock while
           the first x slices load
  chain    per-p normal form M^8 = V rR(th) V^{-1}: twiddle by unit
           rotations, hardware prefix scan (tensor_tensor_scan, per-p real
           multiplier r), un-twiddle into S''=R(th i)v(i); all on DVE,
           aligned step-1 fp16; the chunk shift S'(i)=S''(i-1) is absorbed
           into the inject matmul rhs offset
  local    out[:, i, tau] += sum_{s<=tau} Phi_s x[:, i, tau-s]: fp8
           DoubleRow lag-pairs; lag-0 (with diag D folded) stays fp16
  inject   out[:, i, tau] += Psi[tau,q] S'[q] in fp16, split into ri-waves
           so each wave starts right after its half of the chain
  out      tau-major fp16 (x32 weight scale removed in the PSUM copy);
           host upcasts + unshuffles

Validated on HW: rel err ~1.4e-2 (threshold 2e-2), 60.4us.
"""
import numpy as np
import ml_dtypes

import concourse.bacc as bacc
import concourse.mybir as mybir
from concourse.tile import TileContext
from concourse import bass_utils

P = 128
H = 128
L = 8192
BSZ = 8
T = 8
C = L // T              # 1024 chunks
SEG = 2
CS = C // SEG           # 512 chunks per segment

F32 = mybir.dt.float32
F16 = mybir.dt.float16
F8 = mybir.dt.float8e4
FP8 = ml_dtypes.float8_e4m3fn
DR = mybir.MatmulPerfMode.DoubleRow

SV = 64.0               # extraction weight scale (removed in hh copy)
SO = 32.0               # local/inject weight scale (removed in out copy)


def _host_params(A_diag, G_diag, dt):
    f = np.float32
    dt_s = (1.0 / (1.0 + np.exp(-dt.astype(np.float64)))).astype(f)
    A = np.maximum(A_diag.astype(f), f(0.0))
    G = np.maximum(G_diag.astype(f), f(0.0))
    dt2 = np.maximum(dt_s * dt_s, f(1e-6))
    s = np.sqrt(f(1.0) + dt_s * G)
    A_low = (f(2.0) + dt_s * G - f(2.0) * s) / dt2
    A_high = (f(2.0) + dt_s * G + f(2.0) * s) / dt2
    A_fin = A_low + np.maximum(A - A_low, f(0)) - np.maximum(A - A_high, f(0))
    S = f(1.0) + dt_s * G
    M11 = f(1.0) / S
    M12 = -(dt_s / S) * A_fin
    M21 = dt_s / S
    M22 = f(1.0) - (dt_s * dt_s / S) * A_fin
    c1 = dt_s / S
    c2 = dt_s * dt_s / S
    M = np.stack([np.stack([M11, M12], -1), np.stack([M21, M22], -1)], -2)
    c = np.stack([c1, c2], -1)
    return M.astype(np.float64), c.astype(np.float64)


def _normal_form(M):
    """Per-p real normal form M = V K V^{-1} with K = r R(th) or Jordan."""
    Pn = M.shape[0]
    V = np.zeros((Pn, 2, 2))
    r = np.zeros(Pn)
    th = np.zeros(Pn)
    kap = np.zeros(Pn)
    for p in range(Pn):
        a, b = M[p, 0, 0], M[p, 0, 1]
        cc, d = M[p, 1, 0], M[p, 1, 1]
        m = 0.5 * (a + d)
        disc = (a - d) ** 2 + 4 * b * cc
        if disc < -1e-12:
            w = 0.5 * np.sqrt(-disc)
            lam = m + 1j * w
            u = np.array([b, lam - a]) if abs(b) > 1e-14 else np.array([lam - d, cc])
            phi = 0.5 * np.angle(u @ u)
            u = u * np.exp(-1j * phi)
            ur, ui = u.real, u.imag
            Vp = np.stack([ur, -ui], axis=1)
            Vp /= np.sqrt(max(np.linalg.norm(ur) * np.linalg.norm(ui), 1e-30))
            V[p] = Vp
            r[p] = np.hypot(m, w)
            th[p] = np.arctan2(w, m)
        else:
            N = M[p] - m * np.eye(2)
            r[p] = m
            if np.linalg.norm(N) < 1e-12:
                V[p] = np.eye(2)
            else:
                j = int(np.argmax(np.linalg.norm(N, axis=0)))
                v2 = np.eye(2)[:, j]
                v1 = N @ v2
                v1 /= np.linalg.norm(v1)
                V[p] = np.stack([v1, v2], axis=1)
            K = np.linalg.solve(V[p], M[p] @ V[p])
            kap[p] = K[0, 1]
        K = np.linalg.solve(V[p], M[p] @ V[p])
        if disc < -1e-12:
            Kx = r[p] * np.array(
                [[np.cos(th[p]), -np.sin(th[p])], [np.sin(th[p]), np.cos(th[p])]])
        else:
            Kx = np.array([[r[p], kap[p]], [0, r[p]]])
        assert np.allclose(K, Kx, atol=2e-6), (p, K, Kx)
    return V, np.linalg.inv(V), r, th, kap


def _host_weights(A_diag, G_diag, dt, B, C_, D):
    M1, c = _host_params(A_diag, G_diag, dt)
    V, Vinv, r1, th1, kap1 = _normal_form(M1)
    rc = r1 ** T
    thc = th1 * T
    kapc = T * r1 ** (T - 1) * kap1      # (rI+kN)^T = r^T I + T r^{T-1} kN

    Bre = B[..., 0].astype(np.float64)
    Bim = B[..., 1].astype(np.float64)
    Cre = C_[..., 0].astype(np.float64)
    Cim = C_[..., 1].astype(np.float64)

    Mp = [np.tile(np.eye(2), (P, 1, 1))]
    for _ in range(T):
        Mp.append(np.einsum('pij,pjk->pik', M1, Mp[-1]))

    # local Phi lag matrices (lhsT = Phi.T)
    K_s = np.stack([np.einsum('pij,pj->pi', Mp[s], c)[:, 1] for s in range(T)])
    PhiT = []
    for s in range(T):
        Phi = (Cre * K_s[s]) @ Bre - (Cim * K_s[s]) @ Bim
        if s == 0:
            Phi = Phi + np.diag(D.astype(np.float64))
        PhiT.append(np.ascontiguousarray(Phi.T))
    # wphi0: lag-0 with D, fp16, x SO
    wphi0 = (PhiT[0] * SO).astype(np.float16)
    # wloc8: [pairs a=2..7: (Phi_a.T | Phi_{a-1}.T) | Phi_1.T] fp8, x SO
    wloc8 = np.empty((H, 6 * 2 * H + H), FP8)
    for a in range(2, 8):
        off = (a - 2) * 2 * H
        wloc8[:, off:off + H] = (PhiT[a] * SO).astype(FP8)
        wloc8[:, off + H:off + 2 * H] = (PhiT[a - 1] * SO).astype(FP8)
    wloc8[:, 12 * H:13 * H] = (PhiT[1] * SO).astype(FP8)

    # extraction: q_j = Vinv M^{7-j} c; stream q=k*2+ri; fp8 j-pairs, x SV
    # slot ((qg*4 + jp)*2 + m): pair (W(2jp,q).T | W(2jp+1,q).T), q = qg+2m
    qj = np.stack([np.einsum('pij,pj->pi', Vinv @ Mp[T - 1 - j], c)
                   for j in range(T)])  # (T,P,2)
    wv8 = np.empty((H, 16 * 2 * P), FP8)
    for qg in range(2):
        Bx = (Bre, Bim)[qg]
        for jp in range(4):
            for m in range(2):
                k = m
                off = ((qg * 4 + jp) * 2 + m) * 2 * P
                for g in range(2):
                    W = Bx * qj[2 * jp + g, :, k][:, None]   # (P,H)
                    wv8[:, off + g * P:off + (g + 1) * P] = \
                        (W.T * SV).astype(FP8)

    # injection: w_tau_k = (M^{tau+1} V)[1,k]; lhsT (P,H) = Psi.T, f16, x SO
    wpsi = np.empty((P, T * 4 * H), np.float16)
    for tau in range(T):
        wtk = np.einsum('pij,pjk->pik', Mp[tau + 1], V)[:, 1, :]  # (P,2)
        for k in range(2):
            for ri, Cx, sgn in ((0, Cre, 1.0), (1, Cim, -1.0)):
                Psi = sgn * Cx * wtk[:, k]            # (H,P)
                sl = (tau * 4 + k * 2 + ri) * H
                wpsi[:, sl:sl + H] = (Psi.T * SO).astype(np.float16)

    ii = np.arange(C)
    twc = np.cos(thc[:, None] * ii[None, :]).astype(np.float16)
    tws = np.sin(thc[:, None] * ii[None, :]).astype(np.float16)
    rcb = np.tile(rc[:, None].astype(np.float32), (1, CS))        # (P,512) f32
    kapv = np.ascontiguousarray(kapc[:, None].astype(np.float32))  # (P,1)

    twpack = np.concatenate([twc, tws], axis=1)
    fpack = np.concatenate([rcb, kapv.astype(np.float32)], axis=1)
    return dict(wv8=np.ascontiguousarray(wv8),
                wloc8=np.ascontiguousarray(wloc8),
                wphi0=np.ascontiguousarray(wphi0),
                wpsi=np.ascontiguousarray(wpsi),
                twpack=np.ascontiguousarray(twpack),
                fpack=np.ascontiguousarray(fpack))


def _build_nc():
    nc = bacc.Bacc("TRN2", target_bir_lowering=False, debug=False, num_devices=8)
    Alu = mybir.AluOpType

    x_d = nc.dram_tensor("x", (H, L), F16, kind="ExternalInput").ap()   # tau-major
    x8_d = nc.dram_tensor("x8", (H, L), F8, kind="ExternalInput").ap()  # same layout
    wv8_d = nc.dram_tensor("wv8", (H, 16 * 2 * P), F8, kind="ExternalInput").ap()
    wloc8_d = nc.dram_tensor("wloc8", (H, 13 * H), F8, kind="ExternalInput").ap()
    wphi0_d = nc.dram_tensor("wphi0", (H, H), F16, kind="ExternalInput").ap()
    wpsi_d = nc.dram_tensor("wpsi", (P, T * 4 * H), F16, kind="ExternalInput").ap()
    twpack_d = nc.dram_tensor("twpack", (P, 2 * C), F16, kind="ExternalInput").ap()
    fpack_d = nc.dram_tensor("fpack", (P, CS + 1), F32, kind="ExternalInput").ap()
    out_d = nc.dram_tensor("out", (H, L), F16, kind="ExternalOutput").ap()

    with TileContext(nc) as tc:
        with (
            tc.tile_pool(name="const", bufs=1) as cp,
            tc.tile_pool(name="tmp", bufs=1) as tp,
            tc.tile_pool(name="ps", bufs=1, space="PSUM") as pp,
        ):
            # ---------- loads ----------
            # scalar ring: wv8 qg0, twiddles, wv8 qg1, fpack, wloc8+wphi0, wpsi
            # sync ring: x8 (4 quarter chunks), then x f16 (2 halves), stores
            wv8_sb = cp.tile([H, 16 * 2 * P], F8, tag="wv8")
            nc.scalar.dma_start(wv8_sb[:, 0:16 * P], wv8_d[:, 0:16 * P])
            x8_sb = cp.tile([H, L], F8, tag="x8")
            for hq in range(4):
                off = hq * 4 * CS
                nc.sync.dma_start(x8_sb[:, off:off + 4 * CS],
                                  x8_d[:, off:off + 4 * CS])
            x_sb = cp.tile([H, L], F16, tag="x")
            for hf in range(2):
                off = hf * 8 * CS
                nc.sync.dma_start(x_sb[:, off:off + 8 * CS],
                                  x_d[:, off:off + 8 * CS])
            twpack = cp.tile([P, 2 * C], F16, tag="twpack")
            nc.scalar.dma_start(twpack[:], twpack_d)
            nc.scalar.dma_start(wv8_sb[:, 16 * P:], wv8_d[:, 16 * P:])
            fpack = cp.tile([P, CS + 1], F32, tag="fpack")
            nc.scalar.dma_start(fpack[:], fpack_d)
            wloc8 = cp.tile([H, 13 * H], F8, tag="wloc8")
            nc.scalar.dma_start(wloc8[:], wloc8_d)
            wphi0 = cp.tile([H, H], F16, tag="wphi0")
            nc.scalar.dma_start(wphi0[:], wphi0_d)
            wpsi_sb = cp.tile([P, T * 4 * H], F16, tag="wpsi")
            nc.scalar.dma_start(wpsi_sb[:], wpsi_d)
            twc = twpack[:, 0:C]
            tws = twpack[:, C:2 * C]
            rcb = fpack[:, 0:CS]
            kap = fpack[:, CS:CS + 1]

            x3 = x_sb[:].rearrange("p (g t c) -> p g t c", g=SEG, t=T)
            x83 = x8_sb[:].rearrange("p (g t c) -> p g t c", g=SEG, t=T)

            # ---------- junk warmup: spin PE HAM while first x slices load --
            wsrc = cp.tile([P, CS + P], F16, tag="wsrc")
            nc.vector.memset(wsrc[:].bitcast(F16), 0.0)
            warm = pp.tile([P, CS], F32, tag="ps7", name="warm")
            for w in range(8):
                nc.tensor.matmul(warm[:], wsrc[:, 0:P], wsrc[:, P:P + CS],
                                 start=True, stop=True)

            # ---------- extraction (PE, fp8 DoubleRow j-pairs) ----------
            hh = [cp.tile([P, C], F16, tag=f"hh{q}", name=f"hh{q}")
                  for q in range(4)]
            for sg in range(SEG):
                for qg in range(2):
                    qs = (qg, qg + 2)
                    pse = {q: pp.tile([P, CS], F32, tag=f"ps{sg * 4 + q}",
                                      name=f"pse{sg}{q}") for q in qs}
                    for jp in range(4):
                        rhs = x83[:, sg, 2 * jp:2 * jp + 2, :]
                        for m, q in enumerate(qs):
                            off = ((qg * 4 + jp) * 2 + m) * 2 * P
                            lhsT = wv8_sb[:, off:off + 2 * P].rearrange(
                                "h (two p) -> h two p", two=2)
                            nc.tensor.matmul(
                                pse[q][:], lhsT, rhs,
                                start=(jp == 0), stop=(jp == 3),
                                perf_mode=DR)
                    for q in qs:
                        nc.scalar.mul(hh[q][:, sg * CS:(sg + 1) * CS],
                                      pse[q][:], 1.0 / SV)

            # ---------- chain (DVE) ----------
            # v scans stored unshifted with +2 col pad: v(i) at phys i+2.
            # S''(i) = R(+th i) v(i) at phys i+2; inject reads S'(i)=S''(i-1)
            # via rhs offset (phys sg*CS+1).
            sp = [[cp.tile([P, C + 2], F16, tag=f"sp{k}{ri}", name=f"sp{k}{ri}")
                   for ri in range(2)] for k in range(2)]
            v1p = [tp.tile([P, C + 2], F16, tag=f"v1p{ri}", name=f"v1p{ri}")
                   for ri in range(2)]
            v2p = [tp.tile([P, C + 2], F16, tag=f"v2p{ri}", name=f"v2p{ri}")
                   for ri in range(2)]
            m1 = tp.tile([P, CS], F16, tag="m1")
            m2 = tp.tile([P, CS], F16, tag="m2")
            vin1 = tp.tile([P, CS], F16, tag="vin1")
            vin2 = tp.tile([P, CS], F16, tag="vin2")
            vin1c = tp.tile([P, CS], F16, tag="vin1c")
            for k in range(2):
                for ri in range(2):
                    nc.vector.memset(sp[k][ri][:, 0:2].bitcast(F16), 0.0)
            for ri in range(2):
                nc.vector.memset(v2p[ri][:, 0:2].bitcast(F16), 0.0)

            for sg in range(SEG):
                for ri in range(2):
                    h1 = hh[0 * 2 + ri]
                    h2 = hh[1 * 2 + ri]
                    sl = slice(sg * CS, (sg + 1) * CS)          # chunk i
                    vw = slice(sg * CS + 2, (sg + 1) * CS + 2)  # v(i)/S''(i)
                    vr1 = slice(sg * CS + 1, (sg + 1) * CS + 1)  # v(i-1)
                    # vin2 = twc*h2 - tws*h1
                    nc.vector.tensor_tensor(m1[:], twc[:, sl], h2[:, sl], Alu.mult)
                    nc.vector.tensor_tensor(m2[:], tws[:, sl], h1[:, sl], Alu.mult)
                    nc.vector.tensor_tensor(vin2[:], m1[:], m2[:], Alu.subtract)
                    init2 = 0.0 if sg == 0 else v2p[ri][:, sg * CS + 1:sg * CS + 2]
                    nc.vector.tensor_tensor_scan(
                        v2p[ri][:, vw], rcb, vin2[:], init2, Alu.mult, Alu.add)
                    # vin1 = twc*h1 + tws*h2 + kap * v2(i-1)
                    nc.vector.tensor_tensor(m1[:], twc[:, sl], h1[:, sl], Alu.mult)
                    nc.vector.tensor_tensor(m2[:], tws[:, sl], h2[:, sl], Alu.mult)
                    nc.vector.tensor_tensor(vin1[:], m1[:], m2[:], Alu.add)
                    nc.vector.scalar_tensor_tensor(
                        vin1c[:], v2p[ri][:, vr1], kap[:, 0:1], vin1[:],
                        Alu.mult, Alu.add)
                    init1 = 0.0 if sg == 0 else v1p[ri][:, sg * CS + 1:sg * CS + 2]
                    nc.vector.tensor_tensor_scan(
                        v1p[ri][:, vw], rcb, vin1c[:], init1, Alu.mult, Alu.add)
                    # un-twiddle: S''(i) = R(+th i) v(i)
                    nc.vector.tensor_tensor(m1[:], twc[:, sl], v1p[ri][:, vw],
                                            Alu.mult)
                    nc.vector.tensor_tensor(m2[:], tws[:, sl], v2p[ri][:, vw],
                                            Alu.mult)
                    nc.vector.tensor_tensor(sp[0][ri][:, vw], m1[:], m2[:],
                                            Alu.subtract)
                    nc.vector.tensor_tensor(m1[:], tws[:, sl], v1p[ri][:, vw],
                                            Alu.mult)
                    nc.vector.tensor_tensor(m2[:], twc[:, sl], v2p[ri][:, vw],
                                            Alu.mult)
                    nc.vector.tensor_tensor(sp[1][ri][:, vw], m1[:], m2[:],
                                            Alu.add)

            # ---------- local (fp8 DR pairs + f16 lag0) + inject (f16) ------
            out_sb = cp.tile([H, L], F16, tag="out")
            for sg in range(SEG):
                psl = [pp.tile([P, CS], F32, tag=f"ps{tau}", name=f"psl{sg}{tau}")
                       for tau in range(T)]
                # DR pairs: weights (Phi_a, Phi_{a-1}) on x window (tau-a, +1);
                # tau's lags {tau..1} partition into pairs at a = tau, tau-2, ..
                for a in range(7, 1, -1):
                    off = (a - 2) * 2 * H
                    lhsT = wloc8[:, off:off + 2 * H].rearrange(
                        "h (two m) -> h two m", two=2)
                    for tau in range(a, T, 2):
                        rhs = x83[:, sg, tau - a:tau - a + 2, :]
                        nc.tensor.matmul(psl[tau][:], lhsT, rhs,
                                         start=(tau == a), stop=False,
                                         perf_mode=DR)
                # Phi_1 fp8 singles (odd tau)
                for tau in (1, 3, 5, 7):
                    nc.tensor.matmul(psl[tau][:], wloc8[:, 12 * H:13 * H],
                                     x83[:, sg, tau - 1, :],
                                     start=(tau == 1), stop=False)
                # lag-0 + diag D, f16
                for tau in range(T):
                    nc.tensor.matmul(psl[tau][:], wphi0[:],
                                     x3[:, sg, tau, :],
                                     start=(tau == 0), stop=False)
                # inject in ri-waves: S'(i) = S''(i-1) -> rhs phys offset +1
                spsl = slice(sg * CS + 1, (sg + 1) * CS + 1)
                for ri in range(2):
                    for tau in range(T):
                        for k in range(2):
                            q = k * 2 + ri
                            sl = (tau * 4 + q) * H
                            nc.tensor.matmul(
                                psl[tau][:], wpsi_sb[:, sl:sl + H],
                                sp[k][ri][:, spsl],
                                start=False, stop=(ri == 1 and k == 1))
                        if ri == 1:
                            off = sg * T * CS + tau * CS
                            dst = out_sb[:, off:off + CS]
                            if sg == 1 and tau % 2 == 1:
                                nc.vector.tensor_scalar_mul(
                                    dst, psl[tau][:], 1.0 / SO)
                            else:
                                nc.scalar.mul(dst, psl[tau][:], 1.0 / SO)
                            # store-issues all on the sync engine (idle after
                            # x loads) so copies and issues pipeline instead
                            # of serializing on the scalar sequencer
                            nc.sync.dma_start(out_d[:, off:off + CS], dst)

    nc.compile()
    return nc


_NC_CACHE = None


def _prep(inputs):
    x = np.asarray(inputs["x"], np.float32)
    wts = _host_weights(
        np.asarray(inputs["A_diag"], np.float32),
        np.asarray(inputs["G_diag"], np.float32),
        np.asarray(inputs["dt"], np.float32),
        np.asarray(inputs["B"], np.float32),
        np.asarray(inputs["C"], np.float32),
        np.asarray(inputs["D"], np.float32))
    # (B,L,H) -> (B,H,SEG,T,CS) seg-major tau-major flat (H, L)
    xt = x.reshape(BSZ, SEG, CS, T, H).transpose(0, 4, 1, 3, 2)
    xt = np.ascontiguousarray(xt.reshape(BSZ, H, L))
    xt16 = xt.astype(np.float16)
    xt8 = xt.astype(FP8)
    return [dict(wts, x=xt16[b], x8=xt8[b]) for b in range(BSZ)]


def kernel(x, A_diag, G_diag, dt, B, C, D):
    global _NC_CACHE
    if _NC_CACHE is None:
        _NC_CACHE = _build_nc()
    in_maps = _prep(dict(x=x, A_diag=A_diag, G_diag=G_diag, dt=dt, B=B, C=C, D=D))
    res = bass_utils.run_bass_kernel_spmd(
        _NC_CACHE, in_maps, core_ids=list(range(BSZ)), trace=False)
    out = np.stack([res.results[b]["out"].astype(np.float32)
                    for b in range(BSZ)], 0)  # (B,H,L) tau-major
    # (B,H, sg,tau,il) -> l = (sg*CS+il)*T + tau
    out = out.reshape(BSZ, H, SEG, T, CS).transpose(0, 2, 4, 3, 1)
    out = np.ascontiguousarray(out.reshape(BSZ, L, H))
    return out


# revision 21
# speedup vs baseline: 1.0584x; 1.0584x over previous
"""Trainium2 Bass kernel for nn_DampedIMEX1Layer (v4).

Math: the per-step 2x2 transition M (per diagonal state p) is constant over
time, so the associative scan is a constant-coefficient linear recurrence.
Per core (= one batch element, data-parallel over 8 cores), chunk T=8 over
L=8192 (C=1024 chunks):

  extract  hhat streams (comp k, re/im) with V^{-1}-folded weights, fp8
           DoubleRow j-pairs (PE); q-streams {0,2} first so the ri=0 chain
           starts early; a short junk warmup spins the PE HAM clock while
           the first x slices load
  chain    per-p normal form M^8 = V rR(th) V^{-1}: twiddle by unit
           rotations, hardware prefix scan (tensor_tensor_scan, per-p real
           multiplier r), un-twiddle into S''=R(th i)v(i); all on DVE,
           aligned step-1 fp16; the chunk shift S'(i)=S''(i-1) is absorbed
           into the inject matmul rhs offset
  local    out[:, i, tau] += sum_{s<=tau} Phi_s x[:, i, tau-s]: fp8
           DoubleRow lag-pairs; lag-0 (with diag D folded) stays fp16
  inject   out[:, i, tau] += Psi[tau,q] S'[q] in fp16, split into ri-waves
           so each wave starts right after its half of the chain
  out      tau-major fp16 (x32 weight scale removed in the PSUM copy);
           host upcasts + unshuffles

Validated on HW: rel err ~1.4e-2 (threshold 2e-2), 60.4us.
"""
import numpy as np
import ml_dtypes

import concourse.bacc as bacc
import concourse.mybir as mybir
from concourse.tile import TileContext
from concourse import bass_utils

P = 128
H = 128
L = 8192
BSZ = 8
T = 8
C = L // T              # 1024 chunks
SEG = 2
CS = C // SEG           # 512 chunks per segment

F32 = mybir.dt.float32
F16 = mybir.dt.float16
F8 = mybir.dt.float8e4
FP8 = ml_dtypes.float8_e4m3fn
DR = mybir.MatmulPerfMode.DoubleRow

SV = 64.0               # extraction weight scale (removed in hh copy)
SO = 32.0               # local/inject weight scale (removed in out copy)


def _host_params(A_diag, G_diag, dt):
    f = np.float32
    dt_s = (1.0 / (1.0 + np.exp(-dt.astype(np.float64)))).astype(f)
    A = np.maximum(A_diag.astype(f), f(0.0))
    G = np.maximum(G_diag.astype(f), f(0.0))
    dt2 = np.maximum(dt_s * dt_s, f(1e-6))
    s = np.sqrt(f(1.0) + dt_s * G)
    A_low = (f(2.0) + dt_s * G - f(2.0) * s) / dt2
    A_high = (f(2.0) + dt_s * G + f(2.0) * s) / dt2
    A_fin = A_low + np.maximum(A - A_low, f(0)) - np.maximum(A - A_high, f(0))
    S = f(1.0) + dt_s * G
    M11 = f(1.0) / S
    M12 = -(dt_s / S) * A_fin
    M21 = dt_s / S
    M22 = f(1.0) - (dt_s * dt_s / S) * A_fin
    c1 = dt_s / S
    c2 = dt_s * dt_s / S
    M = np.stack([np.stack([M11, M12], -1), np.stack([M21, M22], -1)], -2)
    c = np.stack([c1, c2], -1)
    return M.astype(np.float64), c.astype(np.float64)


def _normal_form(M):
    """Per-p real normal form M = V K V^{-1} with K = r R(th) or Jordan."""
    Pn = M.shape[0]
    V = np.zeros((Pn, 2, 2))
    r = np.zeros(Pn)
    th = np.zeros(Pn)
    kap = np.zeros(Pn)
    for p in range(Pn):
        a, b = M[p, 0, 0], M[p, 0, 1]
        cc, d = M[p, 1, 0], M[p, 1, 1]
        m = 0.5 * (a + d)
        disc = (a - d) ** 2 + 4 * b * cc
        if disc < -1e-12:
            w = 0.5 * np.sqrt(-disc)
            lam = m + 1j * w
            u = np.array([b, lam - a]) if abs(b) > 1e-14 else np.array([lam - d, cc])
            phi = 0.5 * np.angle(u @ u)
            u = u * np.exp(-1j * phi)
            ur, ui = u.real, u.imag
            Vp = np.stack([ur, -ui], axis=1)
            Vp /= np.sqrt(max(np.linalg.norm(ur) * np.linalg.norm(ui), 1e-30))
            V[p] = Vp
            r[p] = np.hypot(m, w)
            th[p] = np.arctan2(w, m)
        else:
            N = M[p] - m * np.eye(2)
            r[p] = m
            if np.linalg.norm(N) < 1e-12:
                V[p] = np.eye(2)
            else:
                j = int(np.argmax(np.linalg.norm(N, axis=0)))
                v2 = np.eye(2)[:, j]
                v1 = N @ v2
                v1 /= np.linalg.norm(v1)
                V[p] = np.stack([v1, v2], axis=1)
            K = np.linalg.solve(V[p], M[p] @ V[p])
            kap[p] = K[0, 1]
        K = np.linalg.solve(V[p], M[p] @ V[p])
        if disc < -1e-12:
            Kx = r[p] * np.array(
                [[np.cos(th[p]), -np.sin(th[p])], [np.sin(th[p]), np.cos(th[p])]])
        else:
            Kx = np.array([[r[p], kap[p]], [0, r[p]]])
        assert np.allclose(K, Kx, atol=2e-6), (p, K, Kx)
    return V, np.linalg.inv(V), r, th, kap


def _host_weights(A_diag, G_diag, dt, B, C_, D):
    M1, c = _host_params(A_diag, G_diag, dt)
    V, Vinv, r1, th1, kap1 = _normal_form(M1)
    rc = r1 ** T
    thc = th1 * T
    kapc = T * r1 ** (T - 1) * kap1      # (rI+kN)^T = r^T I + T r^{T-1} kN

    Bre = B[..., 0].astype(np.float64)
    Bim = B[..., 1].astype(np.float64)
    Cre = C_[..., 0].astype(np.float64)
    Cim = C_[..., 1].astype(np.float64)

    Mp = [np.tile(np.eye(2), (P, 1, 1))]
    for _ in range(T):
        Mp.append(np.einsum('pij,pjk->pik', M1, Mp[-1]))

    # local Phi lag matrices (lhsT = Phi.T)
    K_s = np.stack([np.einsum('pij,pj->pi', Mp[s], c)[:, 1] for s in range(T)])
    PhiT = []
    for s in range(T):
        Phi = (Cre * K_s[s]) @ Bre - (Cim * K_s[s]) @ Bim
        if s == 0:
            Phi = Phi + np.diag(D.astype(np.float64))
        PhiT.append(np.ascontiguousarray(Phi.T))
    # wphi0: lag-0 with D, fp16, x SO
    wphi0 = (PhiT[0] * SO).astype(np.float16)
    # wloc8: [pairs a=2..7: (Phi_a.T | Phi_{a-1}.T) | Phi_1.T] fp8, x SO
    wloc8 = np.empty((H, 6 * 2 * H + H), FP8)
    for a in range(2, 8):
        off = (a - 2) * 2 * H
        wloc8[:, off:off + H] = (PhiT[a] * SO).astype(FP8)
        wloc8[:, off + H:off + 2 * H] = (PhiT[a - 1] * SO).astype(FP8)
    wloc8[:, 12 * H:13 * H] = (PhiT[1] * SO).astype(FP8)

    # extraction: q_j = Vinv M^{7-j} c; stream q=k*2+ri; fp8 j-pairs, x SV
    # slot ((qg*4 + jp)*2 + m): pair (W(2jp,q).T | W(2jp+1,q).T), q = qg+2m
    qj = np.stack([np.einsum('pij,pj->pi', Vinv @ Mp[T - 1 - j], c)
                   for j in range(T)])  # (T,P,2)
    wv8 = np.empty((H, 16 * 2 * P), FP8)
    for qg in range(2):
        Bx = (Bre, Bim)[qg]
        for jp in range(4):
            for m in range(2):
                k = m
                off = ((qg * 4 + jp) * 2 + m) * 2 * P
                for g in range(2):
                    W = Bx * qj[2 * jp + g, :, k][:, None]   # (P,H)
                    wv8[:, off + g * P:off + (g + 1) * P] = \
                        (W.T * SV).astype(FP8)

    # injection: w_tau_k = (M^{tau+1} V)[1,k]; lhsT (P,H) = Psi.T, f16, x SO
    wpsi = np.empty((P, T * 4 * H), np.float16)
    for tau in range(T):
        wtk = np.einsum('pij,pjk->pik', Mp[tau + 1], V)[:, 1, :]  # (P,2)
        for k in range(2):
            for ri, Cx, sgn in ((0, Cre, 1.0), (1, Cim, -1.0)):
                Psi = sgn * Cx * wtk[:, k]            # (H,P)
                sl = (tau * 4 + k * 2 + ri) * H
                wpsi[:, sl:sl + H] = (Psi.T * SO).astype(np.float16)

    ii = np.arange(C)
    twc = np.cos(thc[:, None] * ii[None, :]).astype(np.float16)
    tws = np.sin(thc[:, None] * ii[None, :]).astype(np.float16)
    rcb = np.tile(rc[:, None].astype(np.float32), (1, CS))        # (P,512) f32
    kapv = np.ascontiguousarray(kapc[:, None].astype(np.float32))  # (P,1)

    twpack = np.concatenate([twc, tws], axis=1)
    fpack = np.concatenate([rcb, kapv.astype(np.float32)], axis=1)
    return dict(wv8=np.ascontiguousarray(wv8),
                wloc8=np.ascontiguousarray(wloc8),
                wphi0=np.ascontiguousarray(wphi0),
                wpsi=np.ascontiguousarray(wpsi),
                twpack=np.ascontiguousarray(twpack),
                fpack=np.ascontiguousarray(fpack))


def _build_nc():
    nc = bacc.Bacc("TRN2", target_bir_lowering=False, debug=False, num_devices=8)
    Alu = mybir.AluOpType

    x_d = nc.dram_tensor("x", (H, L), F16, kind="ExternalInput").ap()   # tau-major
    x8_d = nc.dram_tensor("x8", (H, L), F8, kind="ExternalInput").ap()  # same layout
    wv8_d = nc.dram_tensor("wv8", (H, 16 * 2 * P), F8, kind="ExternalInput").ap()
    wloc8_d = nc.dram_tensor("wloc8", (H, 13 * H), F8, kind="ExternalInput").ap()
    wphi0_d = nc.dram_tensor("wphi0", (H, H), F16, kind="ExternalInput").ap()
    wpsi_d = nc.dram_tensor("wpsi", (P, T * 4 * H), F16, kind="ExternalInput").ap()
    twpack_d = nc.dram_tensor("twpack", (P, 2 * C), F16, kind="ExternalInput").ap()
    fpack_d = nc.dram_tensor("fpack", (P, CS + 1), F32, kind="ExternalInput").ap()
    out_d = nc.dram_tensor("out", (H, L), F16, kind="ExternalOutput").ap()

    with TileContext(nc) as tc:
        with (
            tc.tile_pool(name="const", bufs=1) as cp,
            tc.tile_pool(name="tmp", bufs=1) as tp,
            tc.tile_pool(name="ps", bufs=1, space="PSUM") as pp,
        ):
            # ---------- loads ----------
            # scalar ring: wv8 qg0, twiddles, wv8 qg1, fpack, wloc8+wphi0, wpsi
            # sync ring: x8 (4 quarter chunks), then x f16 (2 halves), stores
            wv8_sb = cp.tile([H, 16 * 2 * P], F8, tag="wv8")
            nc.scalar.dma_start(wv8_sb[:, 0:16 * P], wv8_d[:, 0:16 * P])
            x8t = [cp.tile([H, 8 * CS], F8, tag=f"x8{sg}", name=f"x8{sg}")
                   for sg in range(2)]
            for sg in range(2):
                nc.sync.dma_start(x8t[sg][:], x8_d[:, sg * 8 * CS:(sg + 1) * 8 * CS])
            x_sb = cp.tile([H, L], F16, tag="x")
            for hf in range(2):
                off = hf * 8 * CS
                nc.sync.dma_start(x_sb[:, off:off + 8 * CS],
                                  x_d[:, off:off + 8 * CS])
            twpack = cp.tile([P, 2 * C], F16, tag="twpack")
            nc.scalar.dma_start(twpack[:], twpack_d)
            nc.scalar.dma_start(wv8_sb[:, 16 * P:], wv8_d[:, 16 * P:])
            fpack = cp.tile([P, CS + 1], F32, tag="fpack")
            nc.scalar.dma_start(fpack[:], fpack_d)
            wloc8 = cp.tile([H, 13 * H], F8, tag="wloc8")
            nc.scalar.dma_start(wloc8[:], wloc8_d)
            wphi0 = cp.tile([H, H], F16, tag="wphi0")
            nc.scalar.dma_start(wphi0[:], wphi0_d)
            wpsi_sb = cp.tile([P, T * 4 * H], F16, tag="wpsi")
            nc.scalar.dma_start(wpsi_sb[:], wpsi_d)
            twc = twpack[:, 0:C]
            tws = twpack[:, C:2 * C]
            rcb = fpack[:, 0:CS]
            kap = fpack[:, CS:CS + 1]

            x3 = x_sb[:].rearrange("p (g t c) -> p g t c", g=SEG, t=T)
            x83s = [x8t[sg][:].rearrange("p (t c) -> p t c", t=T)
                    for sg in range(2)]

            # ---------- junk warmup: spin PE HAM while first x slices load --
            wsrc = cp.tile([P, CS + P], F16, tag="wsrc")
            nc.vector.memset(wsrc[:].bitcast(F16), 0.0)
            warm = pp.tile([P, CS], F32, tag="ps7", name="warm")
            for w in range(8):
                nc.tensor.matmul(warm[:], wsrc[:, 0:P], wsrc[:, P:P + CS],
                                 start=True, stop=True)

            # ---------- extraction (PE, fp8 DoubleRow j-pairs) ----------
            hh = [cp.tile([P, C], F16, tag=f"hh{q}", name=f"hh{q}")
                  for q in range(4)]
            for sg in range(SEG):
                for qg in range(2):
                    qs = (qg, qg + 2)
                    pse = {q: pp.tile([P, CS], F32, tag=f"ps{sg * 4 + q}",
                                      name=f"pse{sg}{q}") for q in qs}
                    for jp in range(4):
                        rhs = x83s[sg][:, 2 * jp:2 * jp + 2, :]
                        for m, q in enumerate(qs):
                            off = ((qg * 4 + jp) * 2 + m) * 2 * P
                            lhsT = wv8_sb[:, off:off + 2 * P].rearrange(
                                "h (two p) -> h two p", two=2)
                            nc.tensor.matmul(
                                pse[q][:], lhsT, rhs,
                                start=(jp == 0), stop=(jp == 3),
                                perf_mode=DR)
                    for q in qs:
                        dst = hh[q][:, sg * CS:(sg + 1) * CS]
                        if sg == 0 and qg == 0 and q == 2:
                            # Vector is idle pre-chain: overlap with scalar's
                            # q0 copy so the chain starts one copy earlier
                            nc.vector.tensor_scalar_mul(dst, pse[q][:], 1.0 / SV)
                        else:
                            nc.scalar.mul(dst, pse[q][:], 1.0 / SV)

            # ---------- chain (DVE) ----------
            # v scans stored unshifted with +2 col pad: v(i) at phys i+2.
            # S''(i) = R(+th i) v(i) at phys i+2; inject reads S'(i)=S''(i-1)
            # via rhs offset (phys sg*CS+1).
            sp = [[cp.tile([P, C + 2], F16, tag=f"sp{k}{ri}", name=f"sp{k}{ri}")
                   for ri in range(2)] for k in range(2)]
            v1p = [tp.tile([P, C + 2], F16, tag=f"v1p{ri}", name=f"v1p{ri}")
                   for ri in range(2)]
            v2p = [tp.tile([P, C + 2], F16, tag=f"v2p{ri}", name=f"v2p{ri}")
                   for ri in range(2)]
            m1 = tp.tile([P, CS], F16, tag="m1")
            m2 = tp.tile([P, CS], F16, tag="m2")
            vin1 = tp.tile([P, CS], F16, tag="vin1")
            vin2 = tp.tile([P, CS], F16, tag="vin2")
            vin1c = tp.tile([P, CS], F16, tag="vin1c")
            for k in range(2):
                for ri in range(2):
                    nc.vector.memset(sp[k][ri][:, 0:2].bitcast(F16), 0.0)
            for ri in range(2):
                nc.vector.memset(v2p[ri][:, 0:2].bitcast(F16), 0.0)

            for sg in range(SEG):
                for ri in range(2):
                    h1 = hh[0 * 2 + ri]
                    h2 = hh[1 * 2 + ri]
                    sl = slice(sg * CS, (sg + 1) * CS)          # chunk i
                    vw = slice(sg * CS + 2, (sg + 1) * CS + 2)  # v(i)/S''(i)
                    vr1 = slice(sg * CS + 1, (sg + 1) * CS + 1)  # v(i-1)
                    # vin2 = twc*h2 - tws*h1
                    nc.vector.tensor_tensor(m1[:], twc[:, sl], h2[:, sl], Alu.mult)
                    nc.vector.tensor_tensor(m2[:], tws[:, sl], h1[:, sl], Alu.mult)
                    nc.vector.tensor_tensor(vin2[:], m1[:], m2[:], Alu.subtract)
                    init2 = 0.0 if sg == 0 else v2p[ri][:, sg * CS + 1:sg * CS + 2]
                    nc.vector.tensor_tensor_scan(
                        v2p[ri][:, vw], rcb, vin2[:], init2, Alu.mult, Alu.add)
                    # vin1 = twc*h1 + tws*h2 + kap * v2(i-1)
                    nc.vector.tensor_tensor(m1[:], twc[:, sl], h1[:, sl], Alu.mult)
                    nc.vector.tensor_tensor(m2[:], tws[:, sl], h2[:, sl], Alu.mult)
                    nc.vector.tensor_tensor(vin1[:], m1[:], m2[:], Alu.add)
                    nc.vector.scalar_tensor_tensor(
                        vin1c[:], v2p[ri][:, vr1], kap[:, 0:1], vin1[:],
                        Alu.mult, Alu.add)
                    init1 = 0.0 if sg == 0 else v1p[ri][:, sg * CS + 1:sg * CS + 2]
                    nc.vector.tensor_tensor_scan(
                        v1p[ri][:, vw], rcb, vin1c[:], init1, Alu.mult, Alu.add)
                    # un-twiddle: S''(i) = R(+th i) v(i)
                    nc.vector.tensor_tensor(m1[:], twc[:, sl], v1p[ri][:, vw],
                                            Alu.mult)
                    nc.vector.tensor_tensor(m2[:], tws[:, sl], v2p[ri][:, vw],
                                            Alu.mult)
                    nc.vector.tensor_tensor(sp[0][ri][:, vw], m1[:], m2[:],
                                            Alu.subtract)
                    nc.vector.tensor_tensor(m1[:], tws[:, sl], v1p[ri][:, vw],
                                            Alu.mult)
                    nc.vector.tensor_tensor(m2[:], twc[:, sl], v2p[ri][:, vw],
                                            Alu.mult)
                    nc.vector.tensor_tensor(sp[1][ri][:, vw], m1[:], m2[:],
                                            Alu.add)

            # ---------- local (fp8 DR pairs + f16 lag0) + inject (f16) ------
            out_sb = cp.tile([H, L], F16, tag="out")
            for sg in range(SEG):
                psl = [pp.tile([P, CS], F32, tag=f"ps{tau}", name=f"psl{sg}{tau}")
                       for tau in range(T)]
                # DR pairs: weights (Phi_a, Phi_{a-1}) on x window (tau-a, +1);
                # tau's lags {tau..1} partition into pairs at a = tau, tau-2, ..
                for a in range(7, 1, -1):
                    off = (a - 2) * 2 * H
                    lhsT = wloc8[:, off:off + 2 * H].rearrange(
                        "h (two m) -> h two m", two=2)
                    for tau in range(a, T, 2):
                        rhs = x83s[sg][:, tau - a:tau - a + 2, :]
                        nc.tensor.matmul(psl[tau][:], lhsT, rhs,
                                         start=(tau == a), stop=False,
                                         perf_mode=DR)
                # Phi_1 fp8 singles (odd tau)
                for tau in (1, 3, 5, 7):
                    nc.tensor.matmul(psl[tau][:], wloc8[:, 12 * H:13 * H],
                                     x83s[sg][:, tau - 1, :],
                                     start=(tau == 1), stop=False)
                # lag-0 + diag D, f16
                for tau in range(T):
                    nc.tensor.matmul(psl[tau][:], wphi0[:],
                                     x3[:, sg, tau, :],
                                     start=(tau == 0), stop=False)
                # inject in ri-waves: S'(i) = S''(i-1) -> rhs phys offset +1
                spsl = slice(sg * CS + 1, (sg + 1) * CS + 1)
                for ri in range(2):
                    for tau in range(T):
                        for k in range(2):
                            q = k * 2 + ri
                            sl = (tau * 4 + q) * H
                            nc.tensor.matmul(
                                psl[tau][:], wpsi_sb[:, sl:sl + H],
                                sp[k][ri][:, spsl],
                                start=False, stop=(ri == 1 and k == 1))
                        if ri == 1:
                            off = sg * T * CS + tau * CS
                            dst = out_sb[:, off:off + CS]
                            if sg == 1 and tau % 2 == 1:
                                nc.vector.tensor_scalar_mul(
                                    dst, psl[tau][:], 1.0 / SO)
                                nc.sync.dma_start(out_d[:, off:off + CS], dst)
                            else:
                                nc.scalar.mul(dst, psl[tau][:], 1.0 / SO)
                                nc.scalar.dma_start(out_d[:, off:off + CS], dst)

    nc.compile()
    return nc


_NC_CACHE = None


def _prep(inputs):
    x = np.asarray(inputs["x"], np.float32)
    wts = _host_weights(
        np.asarray(inputs["A_diag"], np.float32),
        np.asarray(inputs["G_diag"], np.float32),
        np.asarray(inputs["dt"], np.float32),
        np.asarray(inputs["B"], np.float32),
        np.asarray(inputs["C"], np.float32),
        np.asarray(inputs["D"], np.float32))
    # (B,L,H) -> (B,H,SEG,T,CS) seg-major tau-major flat (H, L)
    xt = x.reshape(BSZ, SEG, CS, T, H).transpose(0, 4, 1, 3, 2)
    xt = np.ascontiguousarray(xt.reshape(BSZ, H, L))
    xt16 = xt.astype(np.float16)
    xt8 = xt.astype(FP8)
    return [dict(wts, x=xt16[b], x8=xt8[b]) for b in range(BSZ)]


def kernel(x, A_diag, G_diag, dt, B, C, D):
    global _NC_CACHE
    if _NC_CACHE is None:
        _NC_CACHE = _build_nc()
    in_maps = _prep(dict(x=x, A_diag=A_diag, G_diag=G_diag, dt=dt, B=B, C=C, D=D))
    res = bass_utils.run_bass_kernel_spmd(
        _NC_CACHE, in_maps, core_ids=list(range(BSZ)), trace=False)
    out = np.stack([res.results[b]["out"].astype(np.float32)
                    for b in range(BSZ)], 0)  # (B,H,L) tau-major
    # (B,H, sg,tau,il) -> l = (sg*CS+il)*T + tau
    out = out.reshape(BSZ, H, SEG, T, CS).transpose(0, 2, 4, 3, 1)
    out = np.ascontiguousarray(out.reshape(BSZ, L, H))
    return out
